# revision 21
# baseline (speedup 1.0000x reference)
"""Multi-head causal attention (B=2, T=2048, H=16, D=64, C=1024) on 8 trn2 cores.

Sharding: tensor-parallel over heads. Each core owns 2 heads (both batches):
  - computes Q^T/K^T/V^T for its heads over all 4096 tokens
  - causal attention in transposed orientation (S^T[k,q]) so no P transpose
  - partial output projection outT_partial[c, t] = Wo_slice^T @ O^T
Host sums the 8 partials (the "all-reduce"), adds bias, transposes back.

v3 scheduling: the emission order software-pipelines the attention inner loop
(scores of ktile k+1 overlap exp of ktile k via a double-buffered PSUM score
tile) and weaves QKV-projection / output-projection / V-transpose work into
the stream as PE filler so the tensor engine never idles while the scalar
engine computes exp (keeps the HAM clock-gate warm). Causal masking is a
single gpsimd affine_select per diagonal ktile. Partial outputs are written
bf16 with 8KB/partition contiguous DMA layouts; host accumulates in fp32.
"""

import sys

sys.path.insert(0, "/opt/trn_rl_repo")

from collections import deque

import ml_dtypes
import numpy as np

import concourse.bacc as bacc
import concourse.mybir as mybir
import concourse.tile as tile
from concourse.bass_utils import run_bass_kernel_spmd

B, T, C = 2, 2048, 1024
H, D = 16, 64
NT = B * T  # 4096 flattened tokens
N_CORES = 8
HPC = H // N_CORES  # 2 heads per core
FPC = HPC * D  # 128 features per core
CT = C // 128  # 8 contraction tiles for projections
TBLK = 512  # token block
NTB = NT // TBLK  # 8 token blocks
QB = T // TBLK  # 4 query blocks per batch
KT = T // 128  # 16 key tiles per batch

F32 = mybir.dt.float32
BF16 = mybir.dt.bfloat16
FP8 = mybir.dt.float8e4


def build_program():
    nc = bacc.Bacc("TRN2", target_bir_lowering=False, debug=False)

    xt_d = nc.declare_dram_parameter("xt", [128, NTB, CT, TBLK], BF16, isOutput=False)
    x8_d = nc.declare_dram_parameter("x8", [128, NTB, CT, TBLK], FP8, isOutput=False)
    wq_d = nc.declare_dram_parameter("wq", [128, CT, FPC], FP8, isOutput=False)
    wk_d = nc.declare_dram_parameter("wk", [128, CT, FPC], FP8, isOutput=False)
    wv_d = nc.declare_dram_parameter("wv", [128, CT, FPC], BF16, isOutput=False)
    wo_d = nc.declare_dram_parameter("wo", [FPC, C], BF16, isOutput=False)
    out_d = nc.declare_dram_parameter("outT", [128, NTB, CT, TBLK], BF16, isOutput=True)

    with tile.TileContext(nc) as tc:
        with (
            tc.tile_pool(name="slabs", bufs=1) as slabs,
            tc.tile_pool(name="xtp", bufs=2) as xtp,
            tc.tile_pool(name="x8p", bufs=2) as x8p,
            tc.tile_pool(name="esp", bufs=4) as esp,
            tc.tile_pool(name="vtp", bufs=2) as vtp,
            tc.tile_pool(name="rinp", bufs=2) as rinp,
            tc.tile_pool(name="outp", bufs=8) as outp,
            tc.tile_pool(name="psS", bufs=2, space="PSUM") as psS,  # 2x2 banks
            tc.tile_pool(name="psO", bufs=1, space="PSUM") as psO,  # 2 banks
            tc.tile_pool(name="psW", bufs=2, space="PSUM") as psW,  # 2x1 bank
        ):
            # ---- persistent slabs
            qT = slabs.tile([128, NT], BF16, tag="qT")  # [2h*64d, t]
            kT = slabs.tile([128, NT], BF16, tag="kT")
            # V natural layout: per ktile_global: [128k, (ones | V_h0 | V_h1 | ones)]
            # PV stationary h0 = [:, ktg, 0:2, :] = [ones|V_h0] -> rowsum rows 0:64, O 64:128
            #               h1 = [:, ktg, 2:4, :] = [V_h1|ones] -> O rows 0:64, rowsum 64:128
            vN = slabs.tile([128, NTB * 4, 4, 64], BF16, tag="vN")
            oN = slabs.tile([128, NT], BF16, tag="oN")  # normalized O^T
            wq_s = slabs.tile([128, CT, FPC], FP8, tag="wq")
            wk_s = slabs.tile([128, CT, FPC], FP8, tag="wk")
            wv_s = slabs.tile([128, CT, FPC], BF16, tag="wv")
            wo_s = slabs.tile([128, C], BF16, tag="wo")  # [f, c]
            ident = slabs.tile([128, 128], BF16, tag="ident")

            # ---- constants
            from concourse.masks import make_identity
            make_identity(nc, ident[:])
            nc.gpsimd.memset(vN[:, :, 0, :], 1.0)
            nc.gpsimd.memset(vN[:, :, 3, :], 1.0)
            # warm the ACT exp table set during the DMA lead-in so the first
            # real exp doesn't pay the ~2.7us ACT_TABLE_LOAD in-chain
            wtab = slabs.tile([128, 1], F32, tag="wtab")
            nc.scalar.activation(
                wtab[:], ident[:, 0:1], mybir.ActivationFunctionType.Exp, scale=1.0
            )

            # ---- weight loads (wq first; xt0 is prefetched between, see stream)
            def load_wq():
                nc.sync.dma_start(wq_s[:], wq_d[:])

            def load_rest_weights():
                nc.sync.dma_start(wk_s[:], wk_d[:])
                nc.sync.dma_start(wv_s[:], wv_d[:])
                nc.sync.dma_start(wo_s[:], wo_d[:])

            # ---- filler work queue: entries are ((q_mark, kv_mark), fn)
            filler = deque()
            late = deque()  # low-priority work (deferred outproj halves)
            q_done = [-1]   # highest tb whose Q projection is emitted
            kv_done = [-1]  # highest tb whose K/V (incl. transposes) is emitted

            def emit_filler(n):
                for _ in range(n):
                    if filler:
                        (qm, kvm), fn = filler.popleft()
                        fn()
                        q_done[0] = max(q_done[0], qm)
                        kv_done[0] = max(kv_done[0], kvm)
                    elif late:
                        late.popleft()()
                    else:
                        return

            def drain_q_until(tb):
                while q_done[0] < tb and filler:
                    emit_filler(1)

            def drain_kv_until(tb):
                while kv_done[0] < tb and filler:
                    emit_filler(1)

            # ---- QKV projection steps for one token block (queued as filler)
            def queue_qkv(tb):
                xt_t = xtp.tile([128, CT, TBLK], BF16, tag="xt", name=f"xt_{tb}")
                x8_t = x8p.tile([128, CT, TBLK], FP8, tag="x8", name=f"x8_{tb}")
                state = {}

                def dma_step():
                    if tb == 0:
                        nc.sync.dma_start(x8_t[:, 0:2], x8_d[:, tb, 0:2])
                        nc.sync.dma_start(x8_t[:, 2:4], x8_d[:, tb, 2:4])
                        nc.sync.dma_start(x8_t[:, 4:8], x8_d[:, tb, 4:8])
                        nc.sync.dma_start(xt_t[:, 0:4], xt_d[:, tb, 0:4])
                        nc.sync.dma_start(xt_t[:, 4:8], xt_d[:, tb, 4:8])
                    else:
                        nc.sync.dma_start(x8_t[:], x8_d[:, tb])
                        nc.sync.dma_start(xt_t[:], xt_d[:, tb])

                filler.append(((-1, -1), dma_step))

                def mm8_step(name, w_s, c0, nc_):
                    # fp8 DoubleRow: 256-deep contraction per instruction
                    def fn():
                        if c0 == 0:
                            state[name] = psW.tile(
                                [128, TBLK], F32, tag="psw", name=f"ps_{name}_{tb}"
                            )
                        ps = state[name]
                        for c in range(c0, c0 + nc_):
                            nc.tensor.matmul(
                                ps[:],
                                w_s[:, 2 * c : 2 * c + 2, :],
                                x8_t[:, 2 * c : 2 * c + 2, :],
                                start=(c == 0),
                                stop=(c == CT // 2 - 1),
                                perf_mode=mybir.MatmulPerfMode.DoubleRow,
                            )
                    return fn

                def mm_step(name, w_s, ct0, nct):
                    def fn():
                        if ct0 == 0:
                            state[name] = psW.tile(
                                [128, TBLK], F32, tag="psw", name=f"ps_{name}_{tb}"
                            )
                        ps = state[name]
                        for ct in range(ct0, ct0 + nct):
                            nc.tensor.matmul(
                                ps[:],
                                w_s[:, ct, :],
                                xt_t[:, ct, :],
                                start=(ct == 0),
                                stop=(ct == CT - 1),
                            )
                    return fn

                def cast_step(name, dstT):
                    def fn():
                        nc.vector.tensor_copy(
                            dstT[:, tb * TBLK : (tb + 1) * TBLK], state[name][:]
                        )
                    return fn

                if tb == 0:
                    filler.append(((-1, -1), mm8_step("q", wq_s, 0, 1)))
                    filler.append(((-1, -1), mm8_step("q", wq_s, 1, 1)))
                    filler.append(((-1, -1), mm8_step("q", wq_s, 2, 2)))
                else:
                    filler.append(((-1, -1), mm8_step("q", wq_s, 0, 2)))
                    filler.append(((-1, -1), mm8_step("q", wq_s, 2, 2)))
                filler.append(((tb, -1), cast_step("q", qT)))
                filler.append(((-1, -1), mm8_step("k", wk_s, 0, 2)))
                filler.append(((-1, -1), mm8_step("k", wk_s, 2, 2)))
                filler.append(((-1, -1), cast_step("k", kT)))
                filler.append(((-1, -1), mm_step("v", wv_s, 0, 4)))
                filler.append(((-1, -1), mm_step("v", wv_s, 4, 4)))

                vt_t = vtp.tile([128, TBLK], BF16, tag="vt", name=f"vt_{tb}")

                def vcast_step():
                    nc.vector.tensor_copy(vt_t[:], state["v"][:])

                filler.append(((-1, -1), vcast_step))

                tps4 = [None]

                def trans_step(sub):
                    def fn():
                        if sub == 0:
                            tps4[0] = psW.tile([128, 4, 128], BF16, tag="psw", name=f"tps4_{tb}")
                        nc.tensor.transpose(
                            tps4[0][:, sub, :],
                            vt_t[:, sub * 128 : (sub + 1) * 128],
                            ident[:],
                        )
                        if sub == 3:
                            nc.vector.tensor_copy(
                                vN[:, tb * 4 : (tb + 1) * 4, 1:3, :],
                                tps4[0][:].rearrange("p a (b c) -> p a b c", b=2),
                            )
                    return fn

                for sub in range(4):
                    filler.append(((-1, tb if sub == 3 else -1), trans_step(sub)))

            # ---- output projection steps for one attention unit (queued as filler)
            copy_rr = [0]

            def queue_outproj(b, qb, spread=False):
                t0 = b * T + qb * TBLK
                tb = b * QB + qb
                ot = outp.tile([128, CT, TBLK], BF16, tag="ot", name=f"ot_{tb}")

                def proj_step(ct):
                    def fn():
                        ops = psW.tile([128, TBLK], F32, tag="psw")
                        nc.tensor.matmul(
                            ops[:],
                            wo_s[:, ct * 128 : (ct + 1) * 128],
                            oN[:, t0 : t0 + TBLK],
                            start=True,
                            stop=True,
                        )
                        on_scalar = (ct % 2 == 1) if spread else (copy_rr[0] % 4 == 3)
                        if on_scalar:
                            nc.scalar.copy(ot[:, ct, :], ops[:])
                        else:
                            nc.vector.tensor_copy(ot[:, ct, :], ops[:])
                        copy_rr[0] += 1
                        if ct == 3:
                            nc.sync.dma_start(out_d[:, tb, 0:4], ot[:, 0:4])
                        elif ct == CT - 1:
                            nc.sync.dma_start(out_d[:, tb, 4:8], ot[:, 4:8])
                    return fn

                for ct in range(4):
                    filler.append(((-1, -1), proj_step(ct)))
                for ct in range(4, CT):
                    late.append(proj_step(ct))

            # ---- attention for one (batch, qblock), software-pipelined
            def attn(b, qb):
                drain_q_until(b * QB + qb)
                t0 = b * T + qb * TBLK
                O_ps = psO.tile([128, HPC, TBLK], F32, tag="O", name=f"O_{b}_{qb}")
                nkt = (qb + 1) * 4
                prev = None

                def scores_exp(kt):
                    s = kt * 128 - qb * TBLK
                    col0 = max(s, 0)
                    sT = psS.tile([128, HPC, TBLK], F32, tag="sT")
                    es = esp.tile([128, HPC, TBLK], BF16, tag="es")
                    for h in range(HPC):
                        hp = h * 64
                        nc.tensor.matmul(
                            sT[:, h, col0:TBLK],
                            kT[hp : hp + 64, b * T + kt * 128 : b * T + (kt + 1) * 128],
                            qT[hp : hp + 64, t0 + col0 : t0 + TBLK],
                            start=True,
                            stop=True,
                        )
                    nc.scalar.activation(
                        es[:, :, col0:TBLK],
                        sT[:, :, col0:TBLK],
                        mybir.ActivationFunctionType.Exp,
                        scale=0.125,
                    )
                    if s >= 0:
                        # zero strictly-above-diagonal: keep es[p,h,col] iff col>=p
                        nc.gpsimd.affine_select(
                            out=es[:, :, col0 : col0 + 128],
                            in_=es[:, :, col0 : col0 + 128],
                            compare_op=mybir.AluOpType.is_ge,
                            fill=0.0,
                            base=0,
                            pattern=[[0, HPC], [1, 128]],
                            channel_multiplier=-1,
                        )
                    return es, col0

                def pv(kt, es, col0):
                    ktg = b * KT + kt
                    for h in range(HPC):
                        vsta = vN[:, ktg, 0:2, :] if h == 0 else vN[:, ktg, 2:4, :]
                        nc.tensor.matmul(
                            O_ps[:, h, col0:TBLK],
                            vsta,
                            es[:, h, col0:TBLK],
                            start=(kt == 0),
                            stop=(kt == nkt - 1),
                        )

                for kt in range(nkt):
                    drain_kv_until(b * QB + kt // 4)
                    cur = (kt, *scores_exp(kt))
                    if prev is not None:
                        pv(*prev)
                    prev = cur
                    emit_filler(3)
                pv(*prev)

                # normalize: O / rowsum (rowsum rows: h0 -> 0:64, h1 -> 64:128)
                rs = rinp.tile([128, TBLK], F32, tag="rs")
                rin = rinp.tile([128, TBLK], F32, tag="rin")
                nc.vector.tensor_copy(rs[0:64, :], O_ps[0:64, 0, :])
                nc.vector.tensor_copy(rs[64:128, :], O_ps[64:128, 1, :])
                nc.vector.reciprocal_approx_fast(rin[:], rs[:])
                nc.vector.tensor_mul(
                    oN[0:64, t0 : t0 + TBLK], O_ps[64:128, 0, :], rin[0:64, :]
                )
                nc.vector.tensor_mul(
                    oN[64:128, t0 : t0 + TBLK], O_ps[0:64, 1, :], rin[64:128, :]
                )
                queue_outproj(b, qb, spread=(b == 1 and qb == 3))

            # ---- the stream
            load_wq()
            for tb in range(NTB):
                queue_qkv(tb)
            # prologue: xt0 DMA + first Q matmuls before remaining weight loads
            emit_filler(2)
            load_rest_weights()
            drain_kv_until(0)
            for b, qb in (
                (0, 0), (0, 1), (0, 2), (1, 0),
                (0, 3), (1, 1), (1, 2), (1, 3),
            ):
                attn(b, qb)
            emit_filler(10**9)
            while late:
                late.popleft()()

    nc.compile()
    return nc


_NC_CACHE = None


def get_program():
    global _NC_CACHE
    if _NC_CACHE is None:
        _NC_CACHE = build_program()
    return _NC_CACHE


def make_in_maps(x, Wq, Wk, Wv, Wo):
    bf = ml_dtypes.bfloat16
    f8 = ml_dtypes.float8_e4m3
    # xt layout [p, tb, ct, t] so each per-tb DMA is 8KB/partition contiguous
    xt_f = np.asarray(x, np.float32).reshape(NT, C).T  # [C, NT]
    xt_r = np.ascontiguousarray(xt_f.reshape(CT, 128, NTB, TBLK).transpose(1, 2, 0, 3))
    xt = xt_r.astype(bf)
    x8 = xt_r.astype(f8)
    wq_b = np.asarray(Wq, np.float32).astype(f8)
    wk_b = np.asarray(Wk, np.float32).astype(f8)
    wv_b = np.asarray(Wv, np.float32).astype(bf)
    wo_b = np.asarray(Wo, np.float32).astype(bf)
    in_maps = []
    for cid in range(N_CORES):
        sl = slice(cid * FPC, (cid + 1) * FPC)
        in_maps.append(
            {
                "xt": xt,
                "x8": x8,
                "wq": np.ascontiguousarray(
                    wq_b[:, sl].reshape(CT, 128, FPC).transpose(1, 0, 2)
                ),
                "wk": np.ascontiguousarray(
                    wk_b[:, sl].reshape(CT, 128, FPC).transpose(1, 0, 2)
                ),
                "wv": np.ascontiguousarray(
                    wv_b[:, sl].reshape(CT, 128, FPC).transpose(1, 0, 2)
                ),
                "wo": np.ascontiguousarray(wo_b[sl, :]),
            }
        )
    return in_maps


def kernel(x, Wq, Wk, Wv, Wo, bo, _trace=False, _tmpdir=None):
    x = np.asarray(x, dtype=np.float32)
    in_maps = make_in_maps(x, Wq, Wk, Wv, Wo)
    nc = get_program()
    res = run_bass_kernel_spmd(
        nc, in_maps, core_ids=list(range(N_CORES)), trace=_trace, tmpdir=_tmpdir
    )
    acc = res.results[0]["outT"].astype(np.float32)
    for i in range(1, N_CORES):
        acc = acc + res.results[i]["outT"].astype(np.float32)
    # acc [p, tb, ct, t] -> outT [C, NT] with c = ct*128+p, t = tb*512+ti
    outT = acc.transpose(2, 0, 1, 3).reshape(C, NT)
    out = outT.T + np.asarray(bo, np.float32)[None, :]
    if _trace:
        kernel._last_results = res
    return out.reshape(B, T, C).astype(np.float32)


# revision 22
# speedup vs baseline: 1.0283x; 1.0283x over previous
"""Multi-head causal attention (B=2, T=2048, H=16, D=64, C=1024) on 8 trn2 cores.

Sharding: tensor-parallel over heads. Each core owns 2 heads (both batches):
  - computes Q^T/K^T/V^T for its heads over all 4096 tokens
  - causal attention in transposed orientation (S^T[k,q]) so no P transpose
  - partial output projection outT_partial[c, t] = Wo_slice^T @ O^T
Host sums the 8 partials (the "all-reduce"), adds bias, transposes back.

v3 scheduling: the emission order software-pipelines the attention inner loop
(scores of ktile k+1 overlap exp of ktile k via a double-buffered PSUM score
tile) and weaves QKV-projection / output-projection / V-transpose work into
the stream as PE filler so the tensor engine never idles while the scalar
engine computes exp (keeps the HAM clock-gate warm). Causal masking is a
single gpsimd affine_select per diagonal ktile. Partial outputs are written
bf16 with 8KB/partition contiguous DMA layouts; host accumulates in fp32.
"""

import sys

sys.path.insert(0, "/opt/trn_rl_repo")

from collections import deque

import ml_dtypes
import numpy as np

import concourse.bacc as bacc
import concourse.mybir as mybir
import concourse.tile as tile
from concourse.bass_utils import run_bass_kernel_spmd

B, T, C = 2, 2048, 1024
H, D = 16, 64
NT = B * T  # 4096 flattened tokens
N_CORES = 8
HPC = H // N_CORES  # 2 heads per core
FPC = HPC * D  # 128 features per core
CT = C // 128  # 8 contraction tiles for projections
TBLK = 512  # token block
NTB = NT // TBLK  # 8 token blocks
QB = T // TBLK  # 4 query blocks per batch
KT = T // 128  # 16 key tiles per batch

F32 = mybir.dt.float32
BF16 = mybir.dt.bfloat16
FP8 = mybir.dt.float8e4


def build_program():
    nc = bacc.Bacc("TRN2", target_bir_lowering=False, debug=False)

    xt_d = nc.declare_dram_parameter("xt", [128, NTB, CT, TBLK], BF16, isOutput=False)
    x8_d = nc.declare_dram_parameter("x8", [128, NTB, CT, TBLK], FP8, isOutput=False)
    wq_d = nc.declare_dram_parameter("wq", [128, CT, FPC], FP8, isOutput=False)
    wk_d = nc.declare_dram_parameter("wk", [128, CT, FPC], FP8, isOutput=False)
    wv_d = nc.declare_dram_parameter("wv", [128, CT, FPC], BF16, isOutput=False)
    wo_d = nc.declare_dram_parameter("wo", [FPC, C], BF16, isOutput=False)
    out_d = nc.declare_dram_parameter("outT", [128, NTB, CT, TBLK], BF16, isOutput=True)

    with tile.TileContext(nc) as tc:
        with (
            tc.tile_pool(name="slabs", bufs=1) as slabs,
            tc.tile_pool(name="xtp", bufs=2) as xtp,
            tc.tile_pool(name="x8p", bufs=2) as x8p,
            tc.tile_pool(name="esp", bufs=4) as esp,
            tc.tile_pool(name="vtp", bufs=2) as vtp,
            tc.tile_pool(name="rinp", bufs=2) as rinp,
            tc.tile_pool(name="outp", bufs=8) as outp,
            tc.tile_pool(name="psS", bufs=2, space="PSUM") as psS,  # 2x2 banks
            tc.tile_pool(name="psO", bufs=1, space="PSUM") as psO,  # 2 banks
            tc.tile_pool(name="psW", bufs=2, space="PSUM") as psW,  # 2x1 bank
        ):
            # ---- persistent slabs
            qT = slabs.tile([128, NT], BF16, tag="qT")  # [2h*64d, t]
            kT = slabs.tile([128, NT], BF16, tag="kT")
            # V natural layout: per ktile_global: [128k, (ones | V_h0 | V_h1 | ones)]
            # PV stationary h0 = [:, ktg, 0:2, :] = [ones|V_h0] -> rowsum rows 0:64, O 64:128
            #               h1 = [:, ktg, 2:4, :] = [V_h1|ones] -> O rows 0:64, rowsum 64:128
            vN = slabs.tile([128, NTB * 4, 4, 64], BF16, tag="vN")
            oN = slabs.tile([128, NT], BF16, tag="oN")  # normalized O^T
            wq_s = slabs.tile([128, CT, FPC], FP8, tag="wq")
            wk_s = slabs.tile([128, CT, FPC], FP8, tag="wk")
            wv_s = slabs.tile([128, CT, FPC], BF16, tag="wv")
            wo_s = slabs.tile([128, C], BF16, tag="wo")  # [f, c]
            ident = slabs.tile([128, 128], BF16, tag="ident")

            # ---- constants
            from concourse.masks import make_identity
            make_identity(nc, ident[:])
            nc.gpsimd.memset(vN[:, :, 0, :], 1.0)
            nc.gpsimd.memset(vN[:, :, 3, :], 1.0)
            # warm the ACT exp table set during the DMA lead-in so the first
            # real exp doesn't pay the ~2.7us ACT_TABLE_LOAD in-chain
            wtab = slabs.tile([128, 1], F32, tag="wtab")
            nc.scalar.activation(
                wtab[:], ident[:, 0:1], mybir.ActivationFunctionType.Exp, scale=1.0
            )

            # ---- weight loads (wq first; xt0 is prefetched between, see stream)
            def load_wq():
                nc.sync.dma_start(wq_s[:], wq_d[:])

            def load_rest_weights():
                nc.sync.dma_start(wk_s[:], wk_d[:])
                nc.sync.dma_start(wv_s[:], wv_d[:])
                nc.sync.dma_start(wo_s[:], wo_d[:])

            # ---- filler work queue: entries are ((q_mark, kv_mark), fn)
            filler = deque()
            late = deque()  # low-priority work (deferred outproj halves)
            q_done = [-1]   # highest tb whose Q projection is emitted
            kv_done = [-1]  # highest tb whose K/V (incl. transposes) is emitted

            def emit_filler(n):
                for _ in range(n):
                    if filler:
                        (qm, kvm), fn = filler.popleft()
                        fn()
                        q_done[0] = max(q_done[0], qm)
                        kv_done[0] = max(kv_done[0], kvm)
                    elif late:
                        late.popleft()()
                    else:
                        return

            def drain_q_until(tb):
                while q_done[0] < tb and filler:
                    emit_filler(1)

            def drain_kv_until(tb):
                while kv_done[0] < tb and filler:
                    emit_filler(1)

            # ---- QKV projection steps for one token block (queued as filler)
            def queue_qkv(tb):
                xt_t = xtp.tile([128, CT, TBLK], BF16, tag="xt", name=f"xt_{tb}")
                x8_t = x8p.tile([128, CT, TBLK], FP8, tag="x8", name=f"x8_{tb}")
                state = {}

                def dma_step():
                    if tb == 0:
                        nc.sync.dma_start(x8_t[:, 0:4], x8_d[:, tb, 0:4])
                        nc.sync.dma_start(x8_t[:, 4:8], x8_d[:, tb, 4:8])
                        nc.sync.dma_start(xt_t[:, 0:4], xt_d[:, tb, 0:4])
                        nc.sync.dma_start(xt_t[:, 4:8], xt_d[:, tb, 4:8])
                    else:
                        nc.sync.dma_start(x8_t[:], x8_d[:, tb])
                        nc.sync.dma_start(xt_t[:], xt_d[:, tb])

                filler.append(((-1, -1), dma_step))

                def mm8_step(name, w_s, c0, nc_):
                    # fp8 DoubleRow: 256-deep contraction per instruction
                    def fn():
                        if c0 == 0:
                            state[name] = psW.tile(
                                [128, TBLK], F32, tag="psw", name=f"ps_{name}_{tb}"
                            )
                        ps = state[name]
                        for c in range(c0, c0 + nc_):
                            nc.tensor.matmul(
                                ps[:],
                                w_s[:, 2 * c : 2 * c + 2, :],
                                x8_t[:, 2 * c : 2 * c + 2, :],
                                start=(c == 0),
                                stop=(c == CT // 2 - 1),
                                perf_mode=mybir.MatmulPerfMode.DoubleRow,
                            )
                    return fn

                def mm_step(name, w_s, ct0, nct):
                    def fn():
                        if ct0 == 0:
                            state[name] = psW.tile(
                                [128, TBLK], F32, tag="psw", name=f"ps_{name}_{tb}"
                            )
                        ps = state[name]
                        for ct in range(ct0, ct0 + nct):
                            nc.tensor.matmul(
                                ps[:],
                                w_s[:, ct, :],
                                xt_t[:, ct, :],
                                start=(ct == 0),
                                stop=(ct == CT - 1),
                            )
                    return fn

                def cast_step(name, dstT):
                    def fn():
                        nc.vector.tensor_copy(
                            dstT[:, tb * TBLK : (tb + 1) * TBLK], state[name][:]
                        )
                    return fn

                filler.append(((-1, -1), mm8_step("q", wq_s, 0, 2)))
                filler.append(((-1, -1), mm8_step("q", wq_s, 2, 2)))
                filler.append(((tb, -1), cast_step("q", qT)))
                filler.append(((-1, -1), mm8_step("k", wk_s, 0, 2)))
                filler.append(((-1, -1), mm8_step("k", wk_s, 2, 2)))
                filler.append(((-1, -1), cast_step("k", kT)))
                filler.append(((-1, -1), mm_step("v", wv_s, 0, 4)))
                filler.append(((-1, -1), mm_step("v", wv_s, 4, 4)))

                vt_t = vtp.tile([128, TBLK], BF16, tag="vt", name=f"vt_{tb}")

                def vcast_step():
                    nc.vector.tensor_copy(vt_t[:], state["v"][:])

                filler.append(((-1, -1), vcast_step))

                tps4 = [None]

                def trans_step(sub):
                    def fn():
                        if sub == 0:
                            tps4[0] = psW.tile([128, 4, 128], BF16, tag="psw", name=f"tps4_{tb}")
                        nc.tensor.transpose(
                            tps4[0][:, sub, :],
                            vt_t[:, sub * 128 : (sub + 1) * 128],
                            ident[:],
                        )
                        if sub == 3:
                            nc.vector.tensor_copy(
                                vN[:, tb * 4 : (tb + 1) * 4, 1:3, :],
                                tps4[0][:].rearrange("p a (b c) -> p a b c", b=2),
                            )
                    return fn

                for sub in range(4):
                    filler.append(((-1, tb if sub == 3 else -1), trans_step(sub)))

            # ---- output projection steps for one attention unit (queued as filler)
            copy_rr = [0]

            def queue_outproj(b, qb, spread=False):
                t0 = b * T + qb * TBLK
                tb = b * QB + qb
                ot = outp.tile([128, CT, TBLK], BF16, tag="ot", name=f"ot_{tb}")

                def proj_step(ct):
                    def fn():
                        ops = psW.tile([128, TBLK], F32, tag="psw")
                        nc.tensor.matmul(
                            ops[:],
                            wo_s[:, ct * 128 : (ct + 1) * 128],
                            oN[:, t0 : t0 + TBLK],
                            start=True,
                            stop=True,
                        )
                        on_scalar = (ct % 2 == 1) if spread else (copy_rr[0] % 4 == 3)
                        if on_scalar:
                            nc.scalar.copy(ot[:, ct, :], ops[:])
                        else:
                            nc.vector.tensor_copy(ot[:, ct, :], ops[:])
                        copy_rr[0] += 1
                        if ct == 3:
                            nc.sync.dma_start(out_d[:, tb, 0:4], ot[:, 0:4])
                        elif ct == CT - 1:
                            nc.sync.dma_start(out_d[:, tb, 4:8], ot[:, 4:8])
                    return fn

                for ct in range(4):
                    filler.append(((-1, -1), proj_step(ct)))
                for ct in range(4, CT):
                    late.append(proj_step(ct))

            # ---- attention for one (batch, qblock), software-pipelined
            def attn(b, qb):
                drain_q_until(b * QB + qb)
                t0 = b * T + qb * TBLK
                O_ps = psO.tile([128, HPC, TBLK], F32, tag="O", name=f"O_{b}_{qb}")
                nkt = (qb + 1) * 4
                prev = None

                def scores_exp(kt):
                    s = kt * 128 - qb * TBLK
                    col0 = max(s, 0)
                    sT = psS.tile([128, HPC, TBLK], F32, tag="sT")
                    es = esp.tile([128, HPC, TBLK], BF16, tag="es")
                    for h in range(HPC):
                        hp = h * 64
                        nc.tensor.matmul(
                            sT[:, h, col0:TBLK],
                            kT[hp : hp + 64, b * T + kt * 128 : b * T + (kt + 1) * 128],
                            qT[hp : hp + 64, t0 + col0 : t0 + TBLK],
                            start=True,
                            stop=True,
                        )
                    nc.scalar.activation(
                        es[:, :, col0:TBLK],
                        sT[:, :, col0:TBLK],
                        mybir.ActivationFunctionType.Exp,
                        scale=0.125,
                    )
                    if s >= 0:
                        # zero strictly-above-diagonal: keep es[p,h,col] iff col>=p
                        nc.gpsimd.affine_select(
                            out=es[:, :, col0 : col0 + 128],
                            in_=es[:, :, col0 : col0 + 128],
                            compare_op=mybir.AluOpType.is_ge,
                            fill=0.0,
                            base=0,
                            pattern=[[0, HPC], [1, 128]],
                            channel_multiplier=-1,
                        )
                    return es, col0

                def pv(kt, es, col0):
                    ktg = b * KT + kt
                    for h in range(HPC):
                        vsta = vN[:, ktg, 0:2, :] if h == 0 else vN[:, ktg, 2:4, :]
                        nc.tensor.matmul(
                            O_ps[:, h, col0:TBLK],
                            vsta,
                            es[:, h, col0:TBLK],
                            start=(kt == 0),
                            stop=(kt == nkt - 1),
                        )

                for kt in range(nkt):
                    drain_kv_until(b * QB + kt // 4)
                    cur = (kt, *scores_exp(kt))
                    if prev is not None:
                        pv(*prev)
                    prev = cur
                    emit_filler(2)
                pv(*prev)

                # normalize: O / rowsum (rowsum rows: h0 -> 0:64, h1 -> 64:128)
                rs = rinp.tile([128, TBLK], F32, tag="rs")
                rin = rinp.tile([128, TBLK], F32, tag="rin")
                nc.vector.tensor_copy(rs[0:64, :], O_ps[0:64, 0, :])
                nc.vector.tensor_copy(rs[64:128, :], O_ps[64:128, 1, :])
                nc.vector.reciprocal_approx_fast(rin[:], rs[:])
                nc.vector.tensor_mul(
                    oN[0:64, t0 : t0 + TBLK], O_ps[64:128, 0, :], rin[0:64, :]
                )
                nc.vector.tensor_mul(
                    oN[64:128, t0 : t0 + TBLK], O_ps[0:64, 1, :], rin[64:128, :]
                )
                queue_outproj(b, qb, spread=(b == 1 and qb == 3))

            # ---- the stream
            load_wq()
            for tb in range(NTB):
                queue_qkv(tb)
            # prologue: xt0 DMA + first Q matmuls before remaining weight loads
            emit_filler(2)
            load_rest_weights()
            drain_kv_until(0)
            for b, qb in (
                (0, 0), (0, 1), (0, 2), (1, 0),
                (0, 3), (1, 1), (1, 2), (1, 3),
            ):
                attn(b, qb)
            emit_filler(10**9)
            while late:
                late.popleft()()

    nc.compile()
    return nc


_NC_CACHE = None


def get_program():
    global _NC_CACHE
    if _NC_CACHE is None:
        _NC_CACHE = build_program()
    return _NC_CACHE


def make_in_maps(x, Wq, Wk, Wv, Wo):
    bf = ml_dtypes.bfloat16
    f8 = ml_dtypes.float8_e4m3
    # xt layout [p, tb, ct, t] so each per-tb DMA is 8KB/partition contiguous
    xt_f = np.asarray(x, np.float32).reshape(NT, C).T  # [C, NT]
    xt_r = np.ascontiguousarray(xt_f.reshape(CT, 128, NTB, TBLK).transpose(1, 2, 0, 3))
    xt = xt_r.astype(bf)
    x8 = xt_r.astype(f8)
    wq_b = np.asarray(Wq, np.float32).astype(f8)
    wk_b = np.asarray(Wk, np.float32).astype(f8)
    wv_b = np.asarray(Wv, np.float32).astype(bf)
    wo_b = np.asarray(Wo, np.float32).astype(bf)
    in_maps = []
    for cid in range(N_CORES):
        sl = slice(cid * FPC, (cid + 1) * FPC)
        in_maps.append(
            {
                "xt": xt,
                "x8": x8,
                "wq": np.ascontiguousarray(
                    wq_b[:, sl].reshape(CT, 128, FPC).transpose(1, 0, 2)
                ),
                "wk": np.ascontiguousarray(
                    wk_b[:, sl].reshape(CT, 128, FPC).transpose(1, 0, 2)
                ),
                "wv": np.ascontiguousarray(
                    wv_b[:, sl].reshape(CT, 128, FPC).transpose(1, 0, 2)
                ),
                "wo": np.ascontiguousarray(wo_b[sl, :]),
            }
        )
    return in_maps


def kernel(x, Wq, Wk, Wv, Wo, bo, _trace=False, _tmpdir=None):
    x = np.asarray(x, dtype=np.float32)
    in_maps = make_in_maps(x, Wq, Wk, Wv, Wo)
    nc = get_program()
    res = run_bass_kernel_spmd(
        nc, in_maps, core_ids=list(range(N_CORES)), trace=_trace, tmpdir=_tmpdir
    )
    acc = res.results[0]["outT"].astype(np.float32)
    for i in range(1, N_CORES):
        acc = acc + res.results[i]["outT"].astype(np.float32)
    # acc [p, tb, ct, t] -> outT [C, NT] with c = ct*128+p, t = tb*512+ti
    outT = acc.transpose(2, 0, 1, 3).reshape(C, NT)
    out = outT.T + np.asarray(bo, np.float32)[None, :]
    if _trace:
        kernel._last_results = res
    return out.reshape(B, T, C).astype(np.float32)


# revision 23
# speedup vs baseline: 1.0470x; 1.0182x over previous
"""Multi-head causal attention (B=2, T=2048, H=16, D=64, C=1024) on 8 trn2 cores.

Sharding: tensor-parallel over heads. Each core owns 2 heads (both batches):
  - computes Q^T/K^T/V^T for its heads over all 4096 tokens
  - causal attention in transposed orientation (S^T[k,q]) so no P transpose
  - partial output projection outT_partial[c, t] = Wo_slice^T @ O^T
Host sums the 8 partials (the "all-reduce"), adds bias, transposes back.

v3 scheduling: the emission order software-pipelines the attention inner loop
(scores of ktile k+1 overlap exp of ktile k via a double-buffered PSUM score
tile) and weaves QKV-projection / output-projection / V-transpose work into
the stream as PE filler so the tensor engine never idles while the scalar
engine computes exp (keeps the HAM clock-gate warm). Causal masking is a
single gpsimd affine_select per diagonal ktile. Partial outputs are written
bf16 with 8KB/partition contiguous DMA layouts; host accumulates in fp32.
"""

import sys

sys.path.insert(0, "/opt/trn_rl_repo")

from collections import deque

import ml_dtypes
import numpy as np

import concourse.bacc as bacc
import concourse.mybir as mybir
import concourse.tile as tile
from concourse.bass_utils import run_bass_kernel_spmd

B, T, C = 2, 2048, 1024
H, D = 16, 64
NT = B * T  # 4096 flattened tokens
N_CORES = 8
HPC = H // N_CORES  # 2 heads per core
FPC = HPC * D  # 128 features per core
CT = C // 128  # 8 contraction tiles for projections
TBLK = 512  # token block
NTB = NT // TBLK  # 8 token blocks
QB = T // TBLK  # 4 query blocks per batch
KT = T // 128  # 16 key tiles per batch

F32 = mybir.dt.float32
BF16 = mybir.dt.bfloat16
FP8 = mybir.dt.float8e4


def build_program():
    nc = bacc.Bacc("TRN2", target_bir_lowering=False, debug=False)

    xt_d = nc.declare_dram_parameter("xt", [128, NTB, CT, TBLK], BF16, isOutput=False)
    x8_d = nc.declare_dram_parameter("x8", [128, NTB, CT, TBLK], FP8, isOutput=False)
    wq_d = nc.declare_dram_parameter("wq", [128, CT, FPC], FP8, isOutput=False)
    wk_d = nc.declare_dram_parameter("wk", [128, CT, FPC], FP8, isOutput=False)
    wv_d = nc.declare_dram_parameter("wv", [128, CT, FPC], BF16, isOutput=False)
    wo_d = nc.declare_dram_parameter("wo", [FPC, C], BF16, isOutput=False)
    out_d = nc.declare_dram_parameter("outT", [128, NTB, CT, TBLK], BF16, isOutput=True)

    with tile.TileContext(nc) as tc:
        with (
            tc.tile_pool(name="slabs", bufs=1) as slabs,
            tc.tile_pool(name="xtp", bufs=2) as xtp,
            tc.tile_pool(name="x8p", bufs=2) as x8p,
            tc.tile_pool(name="esp", bufs=4) as esp,
            tc.tile_pool(name="vtp", bufs=2) as vtp,
            tc.tile_pool(name="rinp", bufs=2) as rinp,
            tc.tile_pool(name="outp", bufs=8) as outp,
            tc.tile_pool(name="psS", bufs=2, space="PSUM") as psS,  # 2x2 banks
            tc.tile_pool(name="psO", bufs=1, space="PSUM") as psO,  # 2 banks
            tc.tile_pool(name="psW", bufs=2, space="PSUM") as psW,  # 2x1 bank
        ):
            # ---- persistent slabs
            qT = slabs.tile([128, NT], BF16, tag="qT")  # [2h*64d, t]
            kT = slabs.tile([128, NT], BF16, tag="kT")
            # V natural layout: per ktile_global: [128k, (ones | V_h0 | V_h1 | ones)]
            # PV stationary h0 = [:, ktg, 0:2, :] = [ones|V_h0] -> rowsum rows 0:64, O 64:128
            #               h1 = [:, ktg, 2:4, :] = [V_h1|ones] -> O rows 0:64, rowsum 64:128
            vN = slabs.tile([128, NTB * 4, 4, 64], BF16, tag="vN")
            oN = slabs.tile([128, NT], BF16, tag="oN")  # normalized O^T
            wq_s = slabs.tile([128, CT, FPC], FP8, tag="wq")
            wk_s = slabs.tile([128, CT, FPC], FP8, tag="wk")
            wv_s = slabs.tile([128, CT, FPC], BF16, tag="wv")
            wo_s = slabs.tile([128, C], BF16, tag="wo")  # [f, c]
            ident = slabs.tile([128, 128], BF16, tag="ident")

            # ---- constants
            from concourse.masks import make_identity
            make_identity(nc, ident[:])
            nc.gpsimd.memset(vN[:, :, 0, :], 1.0)
            nc.gpsimd.memset(vN[:, :, 3, :], 1.0)
            # warm the ACT exp table set during the DMA lead-in so the first
            # real exp doesn't pay the ~2.7us ACT_TABLE_LOAD in-chain
            wtab = slabs.tile([128, 1], F32, tag="wtab")
            nc.scalar.activation(
                wtab[:], ident[:, 0:1], mybir.ActivationFunctionType.Exp, scale=1.0
            )

            # ---- weight loads (wq first; xt0 is prefetched between, see stream)
            def load_wq():
                nc.sync.dma_start(wq_s[:], wq_d[:])

            def load_rest_weights():
                nc.sync.dma_start(wk_s[:], wk_d[:])
                nc.sync.dma_start(wv_s[:], wv_d[:])
                nc.sync.dma_start(wo_s[:], wo_d[:])

            # ---- filler work queue: entries are ((q_mark, kv_mark), fn)
            filler = deque()
            late = deque()  # low-priority work (deferred outproj halves)
            q_done = [-1]   # highest tb whose Q projection is emitted
            kv_done = [-1]  # highest tb whose K/V (incl. transposes) is emitted

            def emit_filler(n):
                for _ in range(n):
                    if filler:
                        (qm, kvm), fn = filler.popleft()
                        fn()
                        q_done[0] = max(q_done[0], qm)
                        kv_done[0] = max(kv_done[0], kvm)
                    elif late:
                        late.popleft()()
                    else:
                        return

            def drain_q_until(tb):
                while q_done[0] < tb and filler:
                    emit_filler(1)

            def drain_kv_until(tb):
                while kv_done[0] < tb and filler:
                    emit_filler(1)

            # ---- QKV projection steps for one token block (queued as filler)
            pending_trans = []

            def queue_qkv(tb):
                xt_t = xtp.tile([128, CT, TBLK], BF16, tag="xt", name=f"xt_{tb}")
                x8_t = x8p.tile([128, CT, TBLK], FP8, tag="x8", name=f"x8_{tb}")
                state = {}

                def dma_step():
                    if tb == 0:
                        nc.sync.dma_start(x8_t[:, 0:4], x8_d[:, tb, 0:4])
                        nc.sync.dma_start(x8_t[:, 4:8], x8_d[:, tb, 4:8])
                        nc.sync.dma_start(xt_t[:, 0:4], xt_d[:, tb, 0:4])
                        nc.sync.dma_start(xt_t[:, 4:8], xt_d[:, tb, 4:8])
                    else:
                        nc.sync.dma_start(x8_t[:], x8_d[:, tb])
                        nc.sync.dma_start(xt_t[:], xt_d[:, tb])

                filler.append(((-1, -1), dma_step))

                def mm8_step(name, w_s, c0, nc_):
                    # fp8 DoubleRow: 256-deep contraction per instruction
                    def fn():
                        if c0 == 0:
                            state[name] = psW.tile(
                                [128, TBLK], F32, tag="psw", name=f"ps_{name}_{tb}"
                            )
                        ps = state[name]
                        for c in range(c0, c0 + nc_):
                            nc.tensor.matmul(
                                ps[:],
                                w_s[:, 2 * c : 2 * c + 2, :],
                                x8_t[:, 2 * c : 2 * c + 2, :],
                                start=(c == 0),
                                stop=(c == CT // 2 - 1),
                                perf_mode=mybir.MatmulPerfMode.DoubleRow,
                            )
                    return fn

                def mm_step(name, w_s, ct0, nct):
                    def fn():
                        if ct0 == 0:
                            state[name] = psW.tile(
                                [128, TBLK], F32, tag="psw", name=f"ps_{name}_{tb}"
                            )
                        ps = state[name]
                        for ct in range(ct0, ct0 + nct):
                            nc.tensor.matmul(
                                ps[:],
                                w_s[:, ct, :],
                                xt_t[:, ct, :],
                                start=(ct == 0),
                                stop=(ct == CT - 1),
                            )
                    return fn

                def cast_step(name, dstT):
                    def fn():
                        nc.vector.tensor_copy(
                            dstT[:, tb * TBLK : (tb + 1) * TBLK], state[name][:]
                        )
                    return fn

                filler.append(((-1, -1), mm8_step("q", wq_s, 0, 2)))
                filler.append(((-1, -1), mm8_step("q", wq_s, 2, 2)))
                filler.append(((tb, -1), cast_step("q", qT)))
                # previous tb's V-transposes go here, far from their vt cast,
                # so the PE doesn't stall on the DVE cast completing
                filler.extend(pending_trans)
                pending_trans.clear()
                filler.append(((-1, -1), mm8_step("k", wk_s, 0, 2)))
                filler.append(((-1, -1), mm8_step("k", wk_s, 2, 2)))
                filler.append(((-1, -1), cast_step("k", kT)))
                filler.append(((-1, -1), mm_step("v", wv_s, 0, 4)))
                filler.append(((-1, -1), mm_step("v", wv_s, 4, 4)))

                vt_t = vtp.tile([128, TBLK], BF16, tag="vt", name=f"vt_{tb}")

                def vcast_step():
                    nc.vector.tensor_copy(vt_t[:], state["v"][:])

                filler.append(((-1, -1), vcast_step))

                tps4 = [None]

                def trans_step(sub):
                    def fn():
                        if sub == 0:
                            tps4[0] = psW.tile([128, 4, 128], BF16, tag="psw", name=f"tps4_{tb}")
                        nc.tensor.transpose(
                            tps4[0][:, sub, :],
                            vt_t[:, sub * 128 : (sub + 1) * 128],
                            ident[:],
                        )
                        if sub == 3:
                            nc.vector.tensor_copy(
                                vN[:, tb * 4 : (tb + 1) * 4, 1:3, :],
                                tps4[0][:].rearrange("p a (b c) -> p a b c", b=2),
                            )
                    return fn

                for sub in range(4):
                    pending_trans.append(((-1, tb if sub == 3 else -1), trans_step(sub)))

            # ---- output projection steps for one attention unit (queued as filler)
            copy_rr = [0]

            def queue_outproj(b, qb, spread=False):
                t0 = b * T + qb * TBLK
                tb = b * QB + qb
                ot = outp.tile([128, CT, TBLK], BF16, tag="ot", name=f"ot_{tb}")

                def proj_step(ct):
                    def fn():
                        ops = psW.tile([128, TBLK], F32, tag="psw")
                        nc.tensor.matmul(
                            ops[:],
                            wo_s[:, ct * 128 : (ct + 1) * 128],
                            oN[:, t0 : t0 + TBLK],
                            start=True,
                            stop=True,
                        )
                        on_scalar = (ct % 2 == 1) if spread else (copy_rr[0] % 4 == 3)
                        if on_scalar:
                            nc.scalar.copy(ot[:, ct, :], ops[:])
                        else:
                            nc.vector.tensor_copy(ot[:, ct, :], ops[:])
                        copy_rr[0] += 1
                        if ct == 3:
                            nc.sync.dma_start(out_d[:, tb, 0:4], ot[:, 0:4])
                        elif ct == CT - 1:
                            nc.sync.dma_start(out_d[:, tb, 4:8], ot[:, 4:8])
                    return fn

                for ct in range(4):
                    filler.append(((-1, -1), proj_step(ct)))
                for ct in range(4, CT):
                    late.append(proj_step(ct))

            # ---- attention for one (batch, qblock), software-pipelined
            def attn(b, qb):
                drain_q_until(b * QB + qb)
                t0 = b * T + qb * TBLK
                O_ps = psO.tile([128, HPC, TBLK], F32, tag="O", name=f"O_{b}_{qb}")
                nkt = (qb + 1) * 4
                prev = None

                def scores_exp(kt):
                    s = kt * 128 - qb * TBLK
                    col0 = max(s, 0)
                    sT = psS.tile([128, HPC, TBLK], F32, tag="sT")
                    es = esp.tile([128, HPC, TBLK], BF16, tag="es")
                    for h in range(HPC):
                        hp = h * 64
                        nc.tensor.matmul(
                            sT[:, h, col0:TBLK],
                            kT[hp : hp + 64, b * T + kt * 128 : b * T + (kt + 1) * 128],
                            qT[hp : hp + 64, t0 + col0 : t0 + TBLK],
                            start=True,
                            stop=True,
                        )
                    nc.scalar.activation(
                        es[:, :, col0:TBLK],
                        sT[:, :, col0:TBLK],
                        mybir.ActivationFunctionType.Exp,
                        scale=0.125,
                    )
                    if s >= 0:
                        # zero strictly-above-diagonal: keep es[p,h,col] iff col>=p
                        nc.gpsimd.affine_select(
                            out=es[:, :, col0 : col0 + 128],
                            in_=es[:, :, col0 : col0 + 128],
                            compare_op=mybir.AluOpType.is_ge,
                            fill=0.0,
                            base=0,
                            pattern=[[0, HPC], [1, 128]],
                            channel_multiplier=-1,
                        )
                    return es, col0

                def pv(kt, es, col0):
                    ktg = b * KT + kt
                    for h in range(HPC):
                        vsta = vN[:, ktg, 0:2, :] if h == 0 else vN[:, ktg, 2:4, :]
                        nc.tensor.matmul(
                            O_ps[:, h, col0:TBLK],
                            vsta,
                            es[:, h, col0:TBLK],
                            start=(kt == 0),
                            stop=(kt == nkt - 1),
                        )

                for kt in range(nkt):
                    drain_kv_until(b * QB + kt // 4)
                    cur = (kt, *scores_exp(kt))
                    if prev is not None:
                        pv(*prev)
                    prev = cur
                    emit_filler(2)
                pv(*prev)

                # normalize: O / rowsum (rowsum rows: h0 -> 0:64, h1 -> 64:128)
                rs = rinp.tile([128, TBLK], F32, tag="rs")
                rin = rinp.tile([128, TBLK], F32, tag="rin")
                nc.vector.tensor_copy(rs[0:64, :], O_ps[0:64, 0, :])
                nc.vector.tensor_copy(rs[64:128, :], O_ps[64:128, 1, :])
                nc.vector.reciprocal_approx_fast(rin[:], rs[:])
                nc.vector.tensor_mul(
                    oN[0:64, t0 : t0 + TBLK], O_ps[64:128, 0, :], rin[0:64, :]
                )
                nc.vector.tensor_mul(
                    oN[64:128, t0 : t0 + TBLK], O_ps[0:64, 1, :], rin[64:128, :]
                )
                queue_outproj(b, qb, spread=(b == 1 and qb == 3))

            # ---- the stream
            load_wq()
            for tb in range(NTB):
                queue_qkv(tb)
            filler.extend(pending_trans)
            pending_trans.clear()
            # prologue: xt0 DMA + first Q matmuls before remaining weight loads
            emit_filler(2)
            load_rest_weights()
            drain_kv_until(0)
            for b, qb in (
                (0, 0), (0, 1), (0, 2), (1, 0),
                (0, 3), (1, 1), (1, 2), (1, 3),
            ):
                attn(b, qb)
            emit_filler(10**9)
            while late:
                late.popleft()()

    nc.compile()
    return nc


_NC_CACHE = None


def get_program():
    global _NC_CACHE
    if _NC_CACHE is None:
        _NC_CACHE = build_program()
    return _NC_CACHE


def make_in_maps(x, Wq, Wk, Wv, Wo):
    bf = ml_dtypes.bfloat16
    f8 = ml_dtypes.float8_e4m3
    # xt layout [p, tb, ct, t] so each per-tb DMA is 8KB/partition contiguous
    xt_f = np.asarray(x, np.float32).reshape(NT, C).T  # [C, NT]
    xt_r = np.ascontiguousarray(xt_f.reshape(CT, 128, NTB, TBLK).transpose(1, 2, 0, 3))
    xt = xt_r.astype(bf)
    x8 = xt_r.astype(f8)
    wq_b = np.asarray(Wq, np.float32).astype(f8)
    wk_b = np.asarray(Wk, np.float32).astype(f8)
    wv_b = np.asarray(Wv, np.float32).astype(bf)
    wo_b = np.asarray(Wo, np.float32).astype(bf)
    in_maps = []
    for cid in range(N_CORES):
        sl = slice(cid * FPC, (cid + 1) * FPC)
        in_maps.append(
            {
                "xt": xt,
                "x8": x8,
                "wq": np.ascontiguousarray(
                    wq_b[:, sl].reshape(CT, 128, FPC).transpose(1, 0, 2)
                ),
                "wk": np.ascontiguousarray(
                    wk_b[:, sl].reshape(CT, 128, FPC).transpose(1, 0, 2)
                ),
                "wv": np.ascontiguousarray(
                    wv_b[:, sl].reshape(CT, 128, FPC).transpose(1, 0, 2)
                ),
                "wo": np.ascontiguousarray(wo_b[sl, :]),
            }
        )
    return in_maps


def kernel(x, Wq, Wk, Wv, Wo, bo, _trace=False, _tmpdir=None):
    x = np.asarray(x, dtype=np.float32)
    in_maps = make_in_maps(x, Wq, Wk, Wv, Wo)
    nc = get_program()
    res = run_bass_kernel_spmd(
        nc, in_maps, core_ids=list(range(N_CORES)), trace=_trace, tmpdir=_tmpdir
    )
    acc = res.results[0]["outT"].astype(np.float32)
    for i in range(1, N_CORES):
        acc = acc + res.results[i]["outT"].astype(np.float32)
    # acc [p, tb, ct, t] -> outT [C, NT] with c = ct*128+p, t = tb*512+ti
    outT = acc.transpose(2, 0, 1, 3).reshape(C, NT)
    out = outT.T + np.asarray(bo, np.float32)[None, :]
    if _trace:
        kernel._last_results = res
    return out.reshape(B, T, C).astype(np.float32)


# revision 24
# speedup vs baseline: 1.0486x; 1.0015x over previous
"""Multi-head causal attention (B=2, T=2048, H=16, D=64, C=1024) on 8 trn2 cores.

Sharding: tensor-parallel over heads. Each core owns 2 heads (both batches):
  - computes Q^T/K^T/V^T for its heads over all 4096 tokens
  - causal attention in transposed orientation (S^T[k,q]) so no P transpose
  - partial output projection outT_partial[c, t] = Wo_slice^T @ O^T
Host sums the 8 partials (the "all-reduce"), adds bias, transposes back.

v3 scheduling: the emission order software-pipelines the attention inner loop
(scores of ktile k+1 overlap exp of ktile k via a double-buffered PSUM score
tile) and weaves QKV-projection / output-projection / V-transpose work into
the stream as PE filler so the tensor engine never idles while the scalar
engine computes exp (keeps the HAM clock-gate warm). Causal masking is a
single gpsimd affine_select per diagonal ktile. Partial outputs are written
bf16 with 8KB/partition contiguous DMA layouts; host accumulates in fp32.
"""

import sys

sys.path.insert(0, "/opt/trn_rl_repo")

from collections import deque

import ml_dtypes
import numpy as np

import concourse.bacc as bacc
import concourse.mybir as mybir
import concourse.tile as tile
from concourse.bass_utils import run_bass_kernel_spmd

B, T, C = 2, 2048, 1024
H, D = 16, 64
NT = B * T  # 4096 flattened tokens
N_CORES = 8
HPC = H // N_CORES  # 2 heads per core
FPC = HPC * D  # 128 features per core
CT = C // 128  # 8 contraction tiles for projections
TBLK = 512  # token block
NTB = NT // TBLK  # 8 token blocks
QB = T // TBLK  # 4 query blocks per batch
KT = T // 128  # 16 key tiles per batch

F32 = mybir.dt.float32
BF16 = mybir.dt.bfloat16
FP8 = mybir.dt.float8e4


def build_program():
    nc = bacc.Bacc("TRN2", target_bir_lowering=False, debug=False)

    xt_d = nc.declare_dram_parameter("xt", [128, NTB, CT, TBLK], BF16, isOutput=False)
    x8_d = nc.declare_dram_parameter("x8", [128, NTB, CT, TBLK], FP8, isOutput=False)
    wq_d = nc.declare_dram_parameter("wq", [128, CT, FPC], FP8, isOutput=False)
    wk_d = nc.declare_dram_parameter("wk", [128, CT, FPC], FP8, isOutput=False)
    wv_d = nc.declare_dram_parameter("wv", [128, CT, FPC], BF16, isOutput=False)
    wo_d = nc.declare_dram_parameter("wo", [FPC, C], BF16, isOutput=False)
    out_d = nc.declare_dram_parameter("outT", [128, NTB, CT, TBLK], BF16, isOutput=True)

    with tile.TileContext(nc) as tc:
        with (
            tc.tile_pool(name="slabs", bufs=1) as slabs,
            tc.tile_pool(name="xtp", bufs=2) as xtp,
            tc.tile_pool(name="x8p", bufs=2) as x8p,
            tc.tile_pool(name="esp", bufs=4) as esp,
            tc.tile_pool(name="vtp", bufs=2) as vtp,
            tc.tile_pool(name="rinp", bufs=2) as rinp,
            tc.tile_pool(name="outp", bufs=8) as outp,
            tc.tile_pool(name="psS", bufs=2, space="PSUM") as psS,  # 2x2 banks
            tc.tile_pool(name="psO", bufs=1, space="PSUM") as psO,  # 2 banks
            tc.tile_pool(name="psW", bufs=2, space="PSUM") as psW,  # 2x1 bank
        ):
            # ---- persistent slabs
            qT = slabs.tile([128, NT], BF16, tag="qT")  # [2h*64d, t]
            kT = slabs.tile([128, NT], BF16, tag="kT")
            # V natural layout: per ktile_global: [128k, (ones | V_h0 | V_h1 | ones)]
            # PV stationary h0 = [:, ktg, 0:2, :] = [ones|V_h0] -> rowsum rows 0:64, O 64:128
            #               h1 = [:, ktg, 2:4, :] = [V_h1|ones] -> O rows 0:64, rowsum 64:128
            vN = slabs.tile([128, NTB * 4, 4, 64], BF16, tag="vN")
            oN = slabs.tile([128, NT], BF16, tag="oN")  # normalized O^T
            wq_s = slabs.tile([128, CT, FPC], FP8, tag="wq")
            wk_s = slabs.tile([128, CT, FPC], FP8, tag="wk")
            wv_s = slabs.tile([128, CT, FPC], BF16, tag="wv")
            wo_s = slabs.tile([128, C], BF16, tag="wo")  # [f, c]
            ident = slabs.tile([128, 128], BF16, tag="ident")

            # ---- constants
            from concourse.masks import make_identity
            make_identity(nc, ident[:])
            nc.gpsimd.memset(vN[:, :, 0, :], 1.0)
            nc.gpsimd.memset(vN[:, :, 3, :], 1.0)
            # warm the ACT exp table set during the DMA lead-in so the first
            # real exp doesn't pay the ~2.7us ACT_TABLE_LOAD in-chain
            wtab = slabs.tile([128, 1], F32, tag="wtab")
            nc.scalar.activation(
                wtab[:], ident[:, 0:1], mybir.ActivationFunctionType.Exp, scale=1.0
            )

            # ---- weight loads (wq first; xt0 is prefetched between, see stream)
            def load_wq():
                nc.sync.dma_start(wq_s[:], wq_d[:])

            def load_wk():
                nc.sync.dma_start(wk_s[:], wk_d[:])

            def load_wvo():
                nc.sync.dma_start(wv_s[:], wv_d[:])
                nc.sync.dma_start(wo_s[:], wo_d[:])

            # ---- filler work queue: entries are ((q_mark, kv_mark), fn)
            filler = deque()
            late = deque()  # low-priority work (deferred outproj halves)
            q_done = [-1]   # highest tb whose Q projection is emitted
            kv_done = [-1]  # highest tb whose K/V (incl. transposes) is emitted

            def emit_filler(n):
                for _ in range(n):
                    if filler:
                        (qm, kvm), fn = filler.popleft()
                        fn()
                        q_done[0] = max(q_done[0], qm)
                        kv_done[0] = max(kv_done[0], kvm)
                    elif late:
                        late.popleft()()
                    else:
                        return

            def drain_q_until(tb):
                while q_done[0] < tb and filler:
                    emit_filler(1)

            def drain_kv_until(tb):
                while kv_done[0] < tb and filler:
                    emit_filler(1)

            # ---- QKV projection steps for one token block (queued as filler)
            pending_trans = []

            def queue_qkv(tb):
                xt_t = xtp.tile([128, CT, TBLK], BF16, tag="xt", name=f"xt_{tb}")
                x8_t = x8p.tile([128, CT, TBLK], FP8, tag="x8", name=f"x8_{tb}")
                state = {}

                def dma_step():
                    if tb == 0:
                        nc.sync.dma_start(x8_t[:, 0:4], x8_d[:, tb, 0:4])
                        nc.sync.dma_start(x8_t[:, 4:8], x8_d[:, tb, 4:8])
                    else:
                        nc.sync.dma_start(x8_t[:], x8_d[:, tb])
                        nc.sync.dma_start(xt_t[:], xt_d[:, tb])

                filler.append(((-1, -1), dma_step))
                if tb == 0:
                    def dma_step2():
                        nc.sync.dma_start(xt_t[:, 0:4], xt_d[:, tb, 0:4])
                        nc.sync.dma_start(xt_t[:, 4:8], xt_d[:, tb, 4:8])

                    filler.append(((-1, -1), dma_step2))

                def mm8_step(name, w_s, c0, nc_):
                    # fp8 DoubleRow: 256-deep contraction per instruction
                    def fn():
                        if c0 == 0:
                            state[name] = psW.tile(
                                [128, TBLK], F32, tag="psw", name=f"ps_{name}_{tb}"
                            )
                        ps = state[name]
                        for c in range(c0, c0 + nc_):
                            nc.tensor.matmul(
                                ps[:],
                                w_s[:, 2 * c : 2 * c + 2, :],
                                x8_t[:, 2 * c : 2 * c + 2, :],
                                start=(c == 0),
                                stop=(c == CT // 2 - 1),
                                perf_mode=mybir.MatmulPerfMode.DoubleRow,
                            )
                    return fn

                def mm_step(name, w_s, ct0, nct):
                    def fn():
                        if ct0 == 0:
                            state[name] = psW.tile(
                                [128, TBLK], F32, tag="psw", name=f"ps_{name}_{tb}"
                            )
                        ps = state[name]
                        for ct in range(ct0, ct0 + nct):
                            nc.tensor.matmul(
                                ps[:],
                                w_s[:, ct, :],
                                xt_t[:, ct, :],
                                start=(ct == 0),
                                stop=(ct == CT - 1),
                            )
                    return fn

                def cast_step(name, dstT):
                    def fn():
                        nc.vector.tensor_copy(
                            dstT[:, tb * TBLK : (tb + 1) * TBLK], state[name][:]
                        )
                    return fn

                filler.append(((-1, -1), mm8_step("q", wq_s, 0, 2)))
                filler.append(((-1, -1), mm8_step("q", wq_s, 2, 2)))
                filler.append(((tb, -1), cast_step("q", qT)))
                # previous tb's V-transposes go here, far from their vt cast,
                # so the PE doesn't stall on the DVE cast completing
                filler.extend(pending_trans)
                pending_trans.clear()
                filler.append(((-1, -1), mm8_step("k", wk_s, 0, 2)))
                filler.append(((-1, -1), mm8_step("k", wk_s, 2, 2)))
                filler.append(((-1, -1), cast_step("k", kT)))
                filler.append(((-1, -1), mm_step("v", wv_s, 0, 4)))
                filler.append(((-1, -1), mm_step("v", wv_s, 4, 4)))

                vt_t = vtp.tile([128, TBLK], BF16, tag="vt", name=f"vt_{tb}")

                def vcast_step():
                    nc.vector.tensor_copy(vt_t[:], state["v"][:])

                filler.append(((-1, -1), vcast_step))

                tps4 = [None]

                def trans_step(sub):
                    def fn():
                        if sub == 0:
                            tps4[0] = psW.tile([128, 4, 128], BF16, tag="psw", name=f"tps4_{tb}")
                        nc.tensor.transpose(
                            tps4[0][:, sub, :],
                            vt_t[:, sub * 128 : (sub + 1) * 128],
                            ident[:],
                        )
                        if sub == 3:
                            nc.vector.tensor_copy(
                                vN[:, tb * 4 : (tb + 1) * 4, 1:3, :],
                                tps4[0][:].rearrange("p a (b c) -> p a b c", b=2),
                            )
                    return fn

                for sub in range(4):
                    pending_trans.append(((-1, tb if sub == 3 else -1), trans_step(sub)))

            # ---- output projection steps for one attention unit (queued as filler)
            copy_rr = [0]

            def queue_outproj(b, qb, spread=False):
                t0 = b * T + qb * TBLK
                tb = b * QB + qb
                ot = outp.tile([128, CT, TBLK], BF16, tag="ot", name=f"ot_{tb}")

                def proj_step(ct):
                    def fn():
                        ops = psW.tile([128, TBLK], F32, tag="psw")
                        nc.tensor.matmul(
                            ops[:],
                            wo_s[:, ct * 128 : (ct + 1) * 128],
                            oN[:, t0 : t0 + TBLK],
                            start=True,
                            stop=True,
                        )
                        on_scalar = (ct % 2 == 1) if spread else (copy_rr[0] % 4 == 3)
                        if on_scalar:
                            nc.scalar.copy(ot[:, ct, :], ops[:])
                        else:
                            nc.vector.tensor_copy(ot[:, ct, :], ops[:])
                        copy_rr[0] += 1
                        if ct == 3:
                            nc.sync.dma_start(out_d[:, tb, 0:4], ot[:, 0:4])
                        elif ct == CT - 1:
                            nc.sync.dma_start(out_d[:, tb, 4:8], ot[:, 4:8])
                    return fn

                for ct in range(4):
                    filler.append(((-1, -1), proj_step(ct)))
                for ct in range(4, CT):
                    late.append(proj_step(ct))

            # ---- attention for one (batch, qblock), software-pipelined
            def attn(b, qb):
                drain_q_until(b * QB + qb)
                t0 = b * T + qb * TBLK
                O_ps = psO.tile([128, HPC, TBLK], F32, tag="O", name=f"O_{b}_{qb}")
                nkt = (qb + 1) * 4
                prev = None

                def scores_exp(kt):
                    s = kt * 128 - qb * TBLK
                    col0 = max(s, 0)
                    sT = psS.tile([128, HPC, TBLK], F32, tag="sT")
                    es = esp.tile([128, HPC, TBLK], BF16, tag="es")
                    for h in range(HPC):
                        hp = h * 64
                        nc.tensor.matmul(
                            sT[:, h, col0:TBLK],
                            kT[hp : hp + 64, b * T + kt * 128 : b * T + (kt + 1) * 128],
                            qT[hp : hp + 64, t0 + col0 : t0 + TBLK],
                            start=True,
                            stop=True,
                        )
                    nc.scalar.activation(
                        es[:, :, col0:TBLK],
                        sT[:, :, col0:TBLK],
                        mybir.ActivationFunctionType.Exp,
                        scale=0.125,
                    )
                    if s >= 0:
                        # zero strictly-above-diagonal: keep es[p,h,col] iff col>=p
                        nc.gpsimd.affine_select(
                            out=es[:, :, col0 : col0 + 128],
                            in_=es[:, :, col0 : col0 + 128],
                            compare_op=mybir.AluOpType.is_ge,
                            fill=0.0,
                            base=0,
                            pattern=[[0, HPC], [1, 128]],
                            channel_multiplier=-1,
                        )
                    return es, col0

                def pv(kt, es, col0):
                    ktg = b * KT + kt
                    for h in range(HPC):
                        vsta = vN[:, ktg, 0:2, :] if h == 0 else vN[:, ktg, 2:4, :]
                        nc.tensor.matmul(
                            O_ps[:, h, col0:TBLK],
                            vsta,
                            es[:, h, col0:TBLK],
                            start=(kt == 0),
                            stop=(kt == nkt - 1),
                        )

                for kt in range(nkt):
                    drain_kv_until(b * QB + kt // 4)
                    cur = (kt, *scores_exp(kt))
                    if prev is not None:
                        pv(*prev)
                    prev = cur
                    emit_filler(2)
                pv(*prev)

                # normalize: O / rowsum (rowsum rows: h0 -> 0:64, h1 -> 64:128)
                rs = rinp.tile([128, TBLK], F32, tag="rs")
                rin = rinp.tile([128, TBLK], F32, tag="rin")
                nc.vector.tensor_copy(rs[0:64, :], O_ps[0:64, 0, :])
                nc.vector.tensor_copy(rs[64:128, :], O_ps[64:128, 1, :])
                nc.vector.reciprocal_approx_fast(rin[:], rs[:])
                nc.vector.tensor_mul(
                    oN[0:64, t0 : t0 + TBLK], O_ps[64:128, 0, :], rin[0:64, :]
                )
                nc.vector.tensor_mul(
                    oN[64:128, t0 : t0 + TBLK], O_ps[0:64, 1, :], rin[64:128, :]
                )
                queue_outproj(b, qb, spread=(b == 1 and qb == 3))

            # ---- the stream
            load_wq()
            for tb in range(NTB):
                queue_qkv(tb)
            filler.extend(pending_trans)
            pending_trans.clear()
            # prologue DMA priority: wq, x8(0), wk, xt(0), wv/wo
            emit_filler(1)
            load_wk()
            emit_filler(2)
            load_wvo()
            drain_kv_until(0)
            for b, qb in (
                (0, 0), (0, 1), (0, 2), (1, 0),
                (0, 3), (1, 1), (1, 2), (1, 3),
            ):
                attn(b, qb)
            emit_filler(10**9)
            while late:
                late.popleft()()

    nc.compile()
    return nc


_NC_CACHE = None


def get_program():
    global _NC_CACHE
    if _NC_CACHE is None:
        _NC_CACHE = build_program()
    return _NC_CACHE


def make_in_maps(x, Wq, Wk, Wv, Wo):
    bf = ml_dtypes.bfloat16
    f8 = ml_dtypes.float8_e4m3
    # xt layout [p, tb, ct, t] so each per-tb DMA is 8KB/partition contiguous
    xt_f = np.asarray(x, np.float32).reshape(NT, C).T  # [C, NT]
    xt_r = np.ascontiguousarray(xt_f.reshape(CT, 128, NTB, TBLK).transpose(1, 2, 0, 3))
    xt = xt_r.astype(bf)
    x8 = xt_r.astype(f8)
    wq_b = np.asarray(Wq, np.float32).astype(f8)
    wk_b = np.asarray(Wk, np.float32).astype(f8)
    wv_b = np.asarray(Wv, np.float32).astype(bf)
    wo_b = np.asarray(Wo, np.float32).astype(bf)
    in_maps = []
    for cid in range(N_CORES):
        sl = slice(cid * FPC, (cid + 1) * FPC)
        in_maps.append(
            {
                "xt": xt,
                "x8": x8,
                "wq": np.ascontiguousarray(
                    wq_b[:, sl].reshape(CT, 128, FPC).transpose(1, 0, 2)
                ),
                "wk": np.ascontiguousarray(
                    wk_b[:, sl].reshape(CT, 128, FPC).transpose(1, 0, 2)
                ),
                "wv": np.ascontiguousarray(
                    wv_b[:, sl].reshape(CT, 128, FPC).transpose(1, 0, 2)
                ),
                "wo": np.ascontiguousarray(wo_b[sl, :]),
            }
        )
    return in_maps


def kernel(x, Wq, Wk, Wv, Wo, bo, _trace=False, _tmpdir=None):
    x = np.asarray(x, dtype=np.float32)
    in_maps = make_in_maps(x, Wq, Wk, Wv, Wo)
    nc = get_program()
    res = run_bass_kernel_spmd(
        nc, in_maps, core_ids=list(range(N_CORES)), trace=_trace, tmpdir=_tmpdir
    )
    acc = res.results[0]["outT"].astype(np.float32)
    for i in range(1, N_CORES):
        acc = acc + res.results[i]["outT"].astype(np.float32)
    # acc [p, tb, ct, t] -> outT [C, NT] with c = ct*128+p, t = tb*512+ti
    outT = acc.transpose(2, 0, 1, 3).reshape(C, NT)
    out = outT.T + np.asarray(bo, np.float32)[None, :]
    if _trace:
        kernel._last_results = res
    return out.reshape(B, T, C).astype(np.float32)
